# revision 20
# baseline (speedup 1.0000x reference)
"""Trainium2 Bass kernel for nn_Attention (qkv+BN -> biased softmax attention -> gelu -> proj+BN).

Sharding: data-parallel over batch B=128 across 8 NeuronCores (16 batches each).
BatchNorm (training-mode) statistics all-reduced across cores (tiny collectives).

v2 design (vs baseline):
  - BN1 stats from x-Gram matrix (Sxx = x^T x, sx = sum x) so the stats
    allreduce is issued early and overlaps the qkv matmul; BN1 q/k affine is
    folded into the PSUM->SBUF copy.
  - v computed directly transposed ([m, dv] per batch) -> no DMA transposes.
  - attention scores 4-head-packed via tile_position (contraction 32 row
    strips); rowsums 4-head col-strip packed; exp over PSUM pairs with one
    strided-AP activation; bias multiply exp(s)*exp(b) on DVE in bf16.
  - softmax reciprocal broadcast via stride-0-partition DMA (no PE bcast).
  - per-batch pipeline: qkv(b+2) issued while attention(b) runs.
  - gelu folds BN1-v (attn rows sum to 1); proj + BN2 as final phase.
"""
import os
import numpy as np
import ml_dtypes

import concourse.bass as bass
import concourse.tile as tile
from concourse import bacc, mybir
from concourse.bass_utils import run_bass_kernel_spmd

NCORES = int(os.environ.get("KERN_NCORES", "8"))
TRACE_SIM = os.environ.get("KERN_TRACE_SIM", "") == "1"
B, N, C = 128, 320, 256
NH, DK, DV = 8, 32, 128
H = NH * (2 * DK + DV)       # 1536
DH = NH * DV                 # 1024
BL = B // 8                  # 16 batches/core
R = BL * N                   # 5120 rows/core
NT = B * N                   # 40960 global rows
EPS = 1e-5
SCALE = DK ** -0.5
FP = mybir.dt.float32
BF = mybir.dt.bfloat16

NHC = H // 128               # 12 h-chunks (0-1 q, 2-3 k, 4-11 v)
NRC = R // 128               # 40 row chunks
MCS = [128, 128, 64]         # chunking of N=320 into m-chunks
MBS = [0, 0, 64]             # partition base per m-chunk
AF = mybir.ActivationFunctionType
OP = mybir.AluOpType


def build_program():
    nc = bacc.Bacc("TRN2", target_bir_lowering=False, debug=False,
                   enable_asserts=False, num_devices=NCORES)
    x_d = nc.dram_tensor("x", [R, C], BF, kind="ExternalInput").ap()
    wqkvT_d = nc.dram_tensor("wqkvT", [C, H], BF, kind="ExternalInput").ap()
    wprojT_d = nc.dram_tensor("wprojT", [DH, C], BF, kind="ExternalInput").ap()
    ebp_d = nc.dram_tensor("ebp", [2, 128, 3840], BF, kind="ExternalInput").ap()
    g1_d = nc.dram_tensor("g1c", [128, NHC], FP, kind="ExternalInput").ap()
    b1_d = nc.dram_tensor("b1c", [128, NHC], FP, kind="ExternalInput").ap()
    g2_d = nc.dram_tensor("g2", [1, C], FP, kind="ExternalInput").ap()
    b2_d = nc.dram_tensor("b2", [1, C], FP, kind="ExternalInput").ap()
    id_d = nc.dram_tensor("ident", [128, 128], BF, kind="ExternalInput").ap()
    y_d = nc.dram_tensor("y", [R, C], FP, kind="ExternalOutput").ap()

    with tile.TileContext(nc, trace_sim=TRACE_SIM) as tc:
        with tc.tile_pool(name="const", bufs=1) as constp, \
             tc.tile_pool(name="dram", bufs=1, space="DRAM") as dramp, \
             tc.tile_pool(name="stat", bufs=1) as statp:

            # ---- constants ----
            wq_sb = [constp.tile([128, H], BF, tag=f"wq{cc}", name=f"wq{cc}")
                     for cc in range(2)]
            for cc in range(2):
                nc.sync.dma_start(wq_sb[cc][:], wqkvT_d[cc * 128:(cc + 1) * 128, :])
            wprojT_sb = constp.tile([128, NH * C], BF)
            for dc in range(NH):
                nc.sync.dma_start(wprojT_sb[:, dc * C:(dc + 1) * C],
                                  wprojT_d[dc * 128:(dc + 1) * 128, :])
            ebp_sb = [constp.tile([128, 3840], BF, tag=f"ebp{hg}", name=f"ebp{hg}")
                      for hg in range(2)]
            for hg in range(2):
                nc.sync.dma_start(ebp_sb[hg][:], ebp_d[hg])
            g1_sb = constp.tile([128, NHC], FP)
            b1_sb = constp.tile([128, NHC], FP)
            g2_sb = constp.tile([1, C], FP)
            b2_sb = constp.tile([1, C], FP)
            id_sb = constp.tile([128, 128], BF)
            nc.sync.dma_start(g1_sb[:], g1_d[:])
            nc.sync.dma_start(b1_sb[:], b1_d[:])
            nc.sync.dma_start(g2_sb[:], g2_d[:])
            nc.sync.dma_start(b2_sb[:], b2_d[:])
            nc.sync.dma_start(id_sb[:], id_d[:])
            ones_c = constp.tile([128, 1], BF)
            nc.vector.memset(ones_c[:], 1.0)
            ones_rf = constp.tile([128, 128], FP)
            nc.vector.memset(ones_rf[:], 1.0)

            xT_sb = [constp.tile([128, R], BF, tag=f"xT{cc}", name=f"xT{cc}")
                     for cc in range(2)]
            alpha1 = statp.tile([128, NHC], FP)
            beta1 = statp.tile([128, NHC], FP)
            gvsum = statp.tile([128, NH], FP)

            # ========== Phase A: x load/cast, Gram stats, transpose ==========
            with tc.tile_pool(name="xa", bufs=1) as xap, \
                 tc.tile_pool(name="pa", bufs=1, space="PSUM") as pap:
                xb = [xap.tile([128, C], BF, tag=f"xb{rc}", name=f"xb{rc}")
                      for rc in range(NRC)]
                sxxp = [pap.tile([128, C], FP, tag=f"sxx{cc}", name=f"sxx{cc}")
                        for cc in range(2)]
                sxp = pap.tile([128, 2], FP, tag="sxp")
                for rc in range(NRC):
                    nc.sync.dma_start(xb[rc][:], x_d[rc * 128:(rc + 1) * 128, :])
                    for cc in range(2):
                        # Gram + colsum accumulation over all row chunks
                        nc.tensor.matmul(
                            sxxp[cc][:], xb[rc][:, cc * 128:(cc + 1) * 128],
                            xb[rc][:], start=(rc == 0), stop=(rc == NRC - 1))
                        nc.tensor.matmul(
                            sxp[:, cc:cc + 1],
                            xb[rc][:, cc * 128:(cc + 1) * 128], ones_c[:],
                            start=(rc == 0), stop=(rc == NRC - 1))

                # stats: ssum[h] = Wq @ sx ; ssq[h] = diag(Wq Sxx Wq^T)
                with tc.tile_pool(name="st2", bufs=1) as st2p, \
                     tc.tile_pool(name="pt2", bufs=2, space="PSUM") as pt2p, \
                     tc.tile_pool(name="pst", bufs=1, space="PSUM") as pstp:
                    sxx_sb = [st2p.tile([128, C], BF, tag=f"sxs{cc}",
                                        name=f"sxs{cc}") for cc in range(2)]
                    sx_sb = st2p.tile([128, 2], BF)
                    for cc in range(2):
                        nc.vector.tensor_copy(sxx_sb[cc][:], sxxp[cc][:])
                    nc.vector.tensor_copy(sx_sb[:], sxp[:])
                    t1_sb = [st2p.tile([128, H], BF, tag=f"t1{cc}",
                                       name=f"t1{cc}") for cc in range(2)]
                    for cco in range(2):
                        for nb in range(3):
                            t1p = pt2p.tile([128, 512], FP, tag="t1p")
                            for cci in range(2):
                                nc.tensor.matmul(
                                    t1p[:],
                                    sxx_sb[cci][:, cco * 128:(cco + 1) * 128],
                                    wq_sb[cci][:, nb * 512:(nb + 1) * 512],
                                    start=(cci == 0), stop=(cci == 1))
                            nc.vector.tensor_copy(
                                t1_sb[cco][:, nb * 512:(nb + 1) * 512], t1p[:])
                    tt_sb = [st2p.tile([128, H], BF, tag=f"tt{cc}",
                                       name=f"tt{cc}") for cc in range(2)]
                    for cc in range(2):
                        nc.vector.tensor_tensor(tt_sb[cc][:], t1_sb[cc][:],
                                                wq_sb[cc][:], OP.mult)
                    tstat = pstp.tile([128, 2 * NHC], FP)
                    for hc in range(NHC):
                        for cc in range(2):
                            nc.tensor.matmul(
                                tstat[:, hc:hc + 1],
                                wq_sb[cc][:, hc * 128:(hc + 1) * 128],
                                sx_sb[:, cc:cc + 1],
                                start=(cc == 0), stop=(cc == 1))
                            nc.tensor.matmul(
                                tstat[:, NHC + hc:NHC + hc + 1],
                                tt_sb[cc][:, hc * 128:(hc + 1) * 128],
                                ones_c[:], start=(cc == 0), stop=(cc == 1))
                    stats = statp.tile([128, 2 * NHC], FP)
                    nc.vector.tensor_copy(stats[:], tstat[:])

                # transposes run while the stats allreduce is in flight
                with tc.tile_pool(name="pt", bufs=4, space="PSUM") as ptp:
                    for rc in range(NRC):
                        for cc in range(2):
                            pt = ptp.tile([128, 128], BF, tag="pt")
                            nc.tensor.transpose(
                                pt[:], xb[rc][:, cc * 128:(cc + 1) * 128], id_sb[:])
                            nc.vector.tensor_copy(
                                xT_sb[cc][:, rc * 128:(rc + 1) * 128], pt[:])

            # allreduce BN1 stats (overlaps nothing PE-critical yet: issued early)
            bounce_i = dramp.tile([128, 2 * NHC], FP, tag="b1i")
            bounce_o = dramp.tile([128, 2 * NHC], FP, tag="b1o")
            nc.sync.dma_start(bounce_i[:], stats[:])
            nc.gpsimd.collective_compute(
                "AllReduce", OP.add,
                replica_groups=[list(range(NCORES))],
                ins=[bounce_i.opt()], outs=[bounce_o.opt()])
            statsg = statp.tile([128, 2 * NHC], FP)
            nc.sync.dma_start(statsg[:], bounce_o[:])

            mean1 = statp.tile([128, NHC], FP)
            var1 = statp.tile([128, NHC], FP)
            tmp1 = statp.tile([128, NHC], FP)
            nc.vector.tensor_scalar(mean1[:], statsg[:, 0:NHC], 1.0 / NT, None, OP.mult)
            nc.vector.tensor_scalar(var1[:], statsg[:, NHC:2 * NHC], 1.0 / NT, None,
                                    OP.mult)
            nc.vector.tensor_tensor(tmp1[:], mean1[:], mean1[:], OP.mult)
            nc.vector.tensor_tensor(var1[:], var1[:], tmp1[:], OP.subtract)
            nc.vector.tensor_scalar(var1[:], var1[:], EPS, None, OP.add)
            nc.scalar.activation(tmp1[:], var1[:], AF.Ln)
            nc.scalar.activation(var1[:], tmp1[:], AF.Exp, scale=-0.5)   # rstd
            nc.vector.tensor_tensor(alpha1[:], g1_sb[:], var1[:], OP.mult)
            nc.vector.tensor_tensor(beta1[:], mean1[:], alpha1[:], OP.mult)
            nc.vector.tensor_tensor(beta1[:], b1_sb[:], beta1[:], OP.subtract)

            # ========== Phase A2+B: per-batch qkv + attention, pipelined =====
            av_sb = [constp.tile([128, R], BF, tag=f"av{h}", name=f"av{h}")
                     for h in range(NH)]
            with tc.tile_pool(name="qkr", bufs=5) as qkrp, \
                 tc.tile_pool(name="vtr", bufs=5) as vtrp, \
                 tc.tile_pool(name="etp", bufs=8) as etp, \
                 tc.tile_pool(name="rrp", bufs=2) as rrp, \
                 tc.tile_pool(name="pqk", bufs=2, space="PSUM") as pqkp, \
                 tc.tile_pool(name="pav", bufs=2, space="PSUM") as pavp, \
                 tc.tile_pool(name="ps5", bufs=2, space="PSUM") as ps5p:

                qk_t = {}   # (b, hc) -> [128, 320] bf16 (BN1 applied)
                vt_t = {}   # b -> [128, 3072] bf16 (m-part, mc*1024 + dv)

                def qkv_batch(b):
                    vt = vtrp.tile([128, 3072], BF, tag="vt", name="vt")
                    for mc in range(3):
                        ms, mb = MCS[mc], MBS[mc]
                        for dh in range(2):
                            p5 = ps5p.tile([128, 512], FP, tag="p5", name="p5")
                            for cc in range(2):
                                nc.tensor.matmul(
                                    p5[mb:mb + ms, :],
                                    xT_sb[cc][:, b * N + mc * 128:
                                              b * N + mc * 128 + ms],
                                    wq_sb[cc][:, 512 + dh * 512:1024 + dh * 512],
                                    start=(cc == 0), stop=(cc == 1),
                                    tile_position=(0, mb))
                            dst = vt[mb:mb + ms,
                                     mc * 1024 + dh * 512:mc * 1024 + (dh + 1) * 512]
                            if dh == 0:
                                nc.scalar.copy(dst, p5[mb:mb + ms, :])
                            else:
                                nc.vector.tensor_copy(dst, p5[mb:mb + ms, :])
                    vt_t[b] = vt
                    for hc in range(4):
                        p5 = ps5p.tile([128, 512], FP, tag="p5", name="p5")
                        for cc in range(2):
                            nc.tensor.matmul(
                                p5[:, 0:N],
                                wq_sb[cc][:, hc * 128:(hc + 1) * 128],
                                xT_sb[cc][:, b * N:(b + 1) * N],
                                start=(cc == 0), stop=(cc == 1))
                        qt = qkrp.tile([128, N], BF, tag=f"qk{hc}", name="qt")
                        if b < 4:
                            # raw copy (no alpha dep) + in-place BN fixup on
                            # gpsimd (whose queue already waits the allreduce),
                            # so neither the PSUM chain nor DVE stalls on it
                            nc.vector.tensor_copy(qt[:], p5[:, 0:N])
                            nc.gpsimd.tensor_scalar(
                                qt[:], qt[:],
                                alpha1[:, hc:hc + 1], beta1[:, hc:hc + 1],
                                OP.mult, OP.add)
                        else:
                            nc.vector.tensor_scalar(
                                qt[:], p5[:, 0:N],
                                alpha1[:, hc:hc + 1], beta1[:, hc:hc + 1],
                                OP.mult, OP.add)
                        qk_t[(b, hc)] = qt

                def attention(b):
                    vt = vt_t.pop(b)
                    for hg in range(2):
                        qc = hg
                        ets = []
                        for mc in range(3):
                            ms, mb = MCS[mc], MBS[mc]
                            prs = []
                            for p in range(2):
                                psq = pqkp.tile([128, 1024], FP, tag="psq",
                                                name="psq")
                                for j in range(2):
                                    qr = 32 * (2 * p + j)
                                    nc.tensor.matmul(
                                        psq[mb:mb + ms, j * 512:j * 512 + N],
                                        qk_t[(b, 2 + qc)][qr:qr + 32,
                                                          mc * 128:mc * 128 + ms],
                                        qk_t[(b, qc)][qr:qr + 32, :],
                                        tile_position=(qr, mb))
                                prs.append(psq)
                            for p in range(2):
                                et = etp.tile([128, 640], BF, tag="et", name="et")
                                src = prs[p][mb:mb + ms, :].rearrange(
                                    "q (two c) -> q two c", two=2)[:, :, 0:N]
                                dst = et[mb:mb + ms, :].rearrange(
                                    "q (two c) -> q two c", two=2)
                                nc.scalar.activation(dst, src, AF.Exp, scale=SCALE)
                                eng = nc.gpsimd if mc == 2 else nc.vector
                                eng.tensor_tensor(
                                    et[mb:mb + ms, :], et[mb:mb + ms, :],
                                    ebp_sb[hg][mb:mb + ms,
                                               mc * 1280 + p * 640:
                                               mc * 1280 + (p + 1) * 640],
                                    OP.mult)
                                ets.append(et)
                        # rowsums issued after all QK work so PE doesn't stall
                        # mid-group on the exp/mult chain
                        rs = pavp.tile([128, N], FP, tag="avrs", name="rs")
                        for mc in range(3):
                            ms, mb = MCS[mc], MBS[mc]
                            for q_ in range(4):
                                et = ets[mc * 2 + q_ // 2]
                                nc.tensor.matmul(
                                    rs[32 * q_:32 * q_ + 1, :],
                                    ones_c[mb:mb + ms, 0:1],
                                    et[mb:mb + ms, (q_ % 2) * N:(q_ % 2 + 1) * N],
                                    start=(mc == 0), stop=(mc == 2),
                                    tile_position=(mb, 32 * q_))
                        rrf = rrp.tile([128, N], FP, tag="rrf", name="rrf")
                        nc.vector.reciprocal_approx_fast(rrf[:], rs[:])
                        for q_ in range(4):
                            h = 4 * hg + q_
                            rb = pavp.tile([128, N], FP, tag="avrs", name="rb")
                            nc.tensor.matmul(
                                rb[:], ones_rf[32 * q_:32 * q_ + 1, :],
                                rrf[32 * q_:32 * q_ + 1, :],
                                tile_position=(32 * q_, 0))
                            av = pavp.tile([128, N], FP, tag="avrs", name="av")
                            for mc in range(3):
                                ms, mb = MCS[mc], MBS[mc]
                                et = ets[mc * 2 + q_ // 2]
                                nc.tensor.matmul(
                                    av[:],
                                    vt[mb:mb + ms, mc * 1024 + h * 128:
                                       mc * 1024 + (h + 1) * 128],
                                    et[mb:mb + ms, (q_ % 2) * N:(q_ % 2 + 1) * N],
                                    start=(mc == 0), stop=(mc == 2),
                                    tile_position=(mb, 0))
                            osl = av_sb[h][:, b * N:(b + 1) * N]
                            if q_ % 2 == 0:
                                nc.scalar.copy(osl, av[:])
                            else:
                                nc.vector.tensor_copy(osl, av[:])
                            nc.vector.tensor_tensor(osl, osl, rb[:], OP.mult)
                    for hc in range(4):
                        del qk_t[(b, hc)]

                for b in range(4):
                    qkv_batch(b)
                for b in range(BL):
                    if b + 4 < BL:
                        qkv_batch(b + 4)
                    attention(b)

            # ========== Phase C: gelu, proj, BN2 ==========
            with tc.tile_pool(name="ppy", bufs=4, space="PSUM") as ppy, \
                 tc.tile_pool(name="pst2", bufs=1, space="PSUM") as pst2, \
                 tc.tile_pool(name="yb", bufs=1) as yp, \
                 tc.tile_pool(name="sc3", bufs=4) as scp3:
                for h in range(NH):
                    nc.scalar.activation(av_sb[h][:], av_sb[h][:], AF.Gelu,
                                         scale=alpha1[:, 4 + h:5 + h],
                                         bias=beta1[:, 4 + h:5 + h],
                                         accum_out=gvsum[:, h:h + 1])
                gvs16 = statp.tile([128, NH], BF)
                nc.vector.tensor_copy(gvs16[:], gvsum[:])

                y16 = yp.tile([128, NRC * C], BF)
                ystat = pst2.tile([1, 2 * C], FP, tag="yst")
                for h in range(NH):
                    nc.tensor.matmul(ystat[0:1, 0:C], gvs16[:, h:h + 1],
                                     wprojT_sb[:, h * C:(h + 1) * C],
                                     start=(h == 0), stop=(h == NH - 1))
                for i in range(NRC):
                    py = ppy.tile([128, C], FP, tag="py")
                    for h in range(NH):
                        nc.tensor.matmul(
                            py[:], av_sb[h][:, i * 128:(i + 1) * 128],
                            wprojT_sb[:, h * C:(h + 1) * C],
                            start=(h == 0), stop=(h == NH - 1))
                    ysl = y16[:, i * C:(i + 1) * C]
                    nc.vector.tensor_copy(ysl, py[:])
                    yq = scp3.tile([128, C], BF, tag="yq")
                    nc.scalar.activation(yq[:], ysl, AF.Square)
                    nc.tensor.matmul(ystat[0:1, C:2 * C], ones_c[:, 0:1], yq[:],
                                     start=(i == 0), stop=(i == NRC - 1))

                st2 = statp.tile([1, 2 * C], FP)
                nc.vector.tensor_copy(st2[:], ystat[:])
                b2i = dramp.tile([1, 2 * C], FP, tag="b2i")
                b2o = dramp.tile([1, 2 * C], FP, tag="b2o")
                nc.sync.dma_start(b2i[:], st2[:])
                nc.gpsimd.collective_compute(
                    "AllReduce", OP.add,
                    replica_groups=[list(range(NCORES))],
                    ins=[b2i.opt()], outs=[b2o.opt()])
                st2g = statp.tile([1, 2 * C], FP)
                nc.sync.dma_start(st2g[:], b2o[:])

                mean2 = statp.tile([1, C], FP)
                var2 = statp.tile([1, C], FP)
                tmp2 = statp.tile([1, C], FP)
                alpha2 = statp.tile([1, C], FP)
                beta2 = statp.tile([1, C], FP)
                nc.vector.tensor_scalar(mean2[:], st2g[:, 0:C], 1.0 / NT, None, OP.mult)
                nc.vector.tensor_scalar(var2[:], st2g[:, C:2 * C], 1.0 / NT, None,
                                        OP.mult)
                nc.vector.tensor_tensor(tmp2[:], mean2[:], mean2[:], OP.mult)
                nc.vector.tensor_tensor(var2[:], var2[:], tmp2[:], OP.subtract)
                nc.vector.tensor_scalar(var2[:], var2[:], EPS, None, OP.add)
                nc.scalar.activation(tmp2[:], var2[:], AF.Ln)
                nc.scalar.activation(var2[:], tmp2[:], AF.Exp, scale=-0.5)  # rstd2
                nc.vector.tensor_tensor(alpha2[:], g2_sb[:], var2[:], OP.mult)
                nc.vector.tensor_tensor(beta2[:], mean2[:], alpha2[:], OP.mult)
                nc.vector.tensor_tensor(beta2[:], b2_sb[:], beta2[:], OP.subtract)

                a2ps = pst2.tile([128, C], FP, tag="a2ps")
                b2ps = pst2.tile([128, C], FP, tag="b2ps")
                nc.tensor.matmul(a2ps[:], ones_rf[0:1, :], alpha2[:])
                nc.tensor.matmul(b2ps[:], ones_rf[0:1, :], beta2[:])
                a2bc = statp.tile([128, C], FP)
                b2bc = statp.tile([128, C], FP)
                nc.vector.tensor_copy(a2bc[:], a2ps[:])
                nc.vector.tensor_copy(b2bc[:], b2ps[:])

                for i in range(NRC):
                    yo = scp3.tile([128, C], FP, tag="yo")
                    eng = nc.gpsimd if i % 4 == 3 else nc.vector
                    eng.tensor_tensor(yo[:], y16[:, i * C:(i + 1) * C],
                                      a2bc[:], OP.mult)
                    eng.tensor_tensor(yo[:], yo[:], b2bc[:], OP.add)
                    nc.sync.dma_start(y_d[i * 128:(i + 1) * 128, :], yo[:])

    nc.compile()
    return nc


_PROG = None


def _get_prog():
    global _PROG
    if _PROG is None:
        _PROG = build_program()
    return _PROG


def _host_prep(x, Wqkv, g1, b1, ab, Wproj, g2, b2, idxs):
    perm = np.empty(H, dtype=np.int64)
    for h in range(NH):
        base = h * (2 * DK + DV)
        perm[DK * h: DK * (h + 1)] = np.arange(base, base + DK)
        perm[NH * DK + DK * h: NH * DK + DK * (h + 1)] = \
            np.arange(base + DK, base + 2 * DK)
        perm[2 * NH * DK + DV * h: 2 * NH * DK + DV * (h + 1)] = \
            np.arange(base + 2 * DK, base + 2 * DK + DV)
    x = np.asarray(x, dtype=np.float32)
    Wqkv = np.asarray(Wqkv, dtype=np.float32)
    wqkvT = np.ascontiguousarray(Wqkv[perm, :].T).astype(ml_dtypes.bfloat16)
    g1c = np.ascontiguousarray(np.asarray(g1, np.float32)[perm].reshape(NHC, 128).T)
    b1c = np.ascontiguousarray(np.asarray(b1, np.float32)[perm].reshape(NHC, 128).T)
    wprojT = np.ascontiguousarray(np.asarray(Wproj, np.float32).T).astype(
        ml_dtypes.bfloat16)                                            # (1024, 256)
    eb = np.exp(np.asarray(ab, np.float32))[:, np.asarray(idxs)]       # (8,320,320)
    ebp = np.zeros((2, 128, 3840), np.float32)
    for hg in range(2):
        for mc in range(3):
            ms, mb = MCS[mc], MBS[mc]
            for p in range(2):
                for j in range(2):
                    h = 4 * hg + 2 * p + j
                    c0 = mc * 1280 + p * 640 + j * 320
                    ebp[hg, mb:mb + ms, c0:c0 + N] = \
                        eb[h, mc * 128:mc * 128 + ms, :]
    common = {
        "wqkvT": wqkvT, "wprojT": wprojT,
        "ebp": ebp.astype(ml_dtypes.bfloat16),
        "g1c": g1c, "b1c": b1c,
        "g2": np.asarray(g2, np.float32).reshape(1, C),
        "b2": np.asarray(b2, np.float32).reshape(1, C),
        "ident": np.eye(128, dtype=np.float32).astype(ml_dtypes.bfloat16),
    }
    in_maps = []
    for c in range(NCORES):
        m = dict(common)
        m["x"] = np.ascontiguousarray(x[c * BL:(c + 1) * BL].reshape(R, C)).astype(
            ml_dtypes.bfloat16)
        in_maps.append(m)
    return in_maps


def _run(in_maps, trace=False):
    nc = _get_prog()
    res = run_bass_kernel_spmd(nc, in_maps, core_ids=list(range(NCORES)),
                               trace=trace)
    out = np.concatenate(
        [np.asarray(res.results[c]["y"]).reshape(BL, N, C) for c in range(NCORES)],
        axis=0)
    return out.astype(np.float32), res


def kernel(**inputs):
    out, _ = _run(_host_prep(**inputs))
    return out


def run_traced(**inputs):
    return _run(_host_prep(**inputs), trace=True)


# revision 24
# speedup vs baseline: 1.2245x; 1.2245x over previous
"""Trainium2 Bass kernel for nn_Attention (qkv+BN -> biased softmax attention -> gelu -> proj+BN).

Sharding: data-parallel over batch B=128 across 8 NeuronCores (16 batches each).
BatchNorm (training-mode) statistics all-reduced across cores (tiny collectives).

v2 design (vs baseline):
  - BN1 stats from x-Gram matrix (Sxx = x^T x, sx = sum x) so the stats
    allreduce is issued early and overlaps the qkv matmul; BN1 q/k affine is
    folded into the PSUM->SBUF copy.
  - v computed directly transposed ([m, dv] per batch) -> no DMA transposes.
  - attention scores 4-head-packed via tile_position (contraction 32 row
    strips); rowsums 4-head col-strip packed; exp over PSUM pairs with one
    strided-AP activation; bias multiply exp(s)*exp(b) on DVE in bf16.
  - softmax reciprocal broadcast via stride-0-partition DMA (no PE bcast).
  - per-batch pipeline: qkv(b+2) issued while attention(b) runs.
  - gelu folds BN1-v (attn rows sum to 1); proj + BN2 as final phase.
"""
import os
import numpy as np
import ml_dtypes

import concourse.bass as bass
import concourse.tile as tile
from concourse import bacc, mybir
from concourse.bass_utils import run_bass_kernel_spmd

NCORES = int(os.environ.get("KERN_NCORES", "8"))
TRACE_SIM = os.environ.get("KERN_TRACE_SIM", "") == "1"
B, N, C = 128, 320, 256
NH, DK, DV = 8, 32, 128
H = NH * (2 * DK + DV)       # 1536
DH = NH * DV                 # 1024
BL = B // 8                  # 16 batches/core
R = BL * N                   # 5120 rows/core
NT = B * N                   # 40960 global rows
EPS = 1e-5
SCALE = DK ** -0.5
FP = mybir.dt.float32
BF = mybir.dt.bfloat16

NHC = H // 128               # 12 h-chunks (0-1 q, 2-3 k, 4-11 v)
NRC = R // 128               # 40 row chunks
MCS = [128, 128, 64]         # chunking of N=320 into m-chunks
MBS = [0, 0, 64]             # partition base per m-chunk
AF = mybir.ActivationFunctionType
OP = mybir.AluOpType


def build_program():
    nc = bacc.Bacc("TRN2", target_bir_lowering=False, debug=False,
                   enable_asserts=False, num_devices=NCORES)
    x_d = nc.dram_tensor("x", [R, C], BF, kind="ExternalInput").ap()
    wqkvT_d = nc.dram_tensor("wqkvT", [C, H], BF, kind="ExternalInput").ap()
    wprojT_d = nc.dram_tensor("wprojT", [DH, C], BF, kind="ExternalInput").ap()
    ebp_d = nc.dram_tensor("ebp", [2, 128, 3840], BF, kind="ExternalInput").ap()
    g1_d = nc.dram_tensor("g1c", [128, NHC], FP, kind="ExternalInput").ap()
    b1_d = nc.dram_tensor("b1c", [128, NHC], FP, kind="ExternalInput").ap()
    g2_d = nc.dram_tensor("g2", [1, C], FP, kind="ExternalInput").ap()
    b2_d = nc.dram_tensor("b2", [1, C], FP, kind="ExternalInput").ap()
    id_d = nc.dram_tensor("ident", [128, 128], BF, kind="ExternalInput").ap()
    y_d = nc.dram_tensor("y", [R, C], FP, kind="ExternalOutput").ap()

    with tile.TileContext(nc, trace_sim=TRACE_SIM) as tc:
        with tc.tile_pool(name="const", bufs=1) as constp, \
             tc.tile_pool(name="dram", bufs=1, space="DRAM") as dramp, \
             tc.tile_pool(name="stat", bufs=1) as statp:

            # ---- constants ----
            wq_sb = [constp.tile([128, H], BF, tag=f"wq{cc}", name=f"wq{cc}")
                     for cc in range(2)]
            for cc in range(2):
                nc.sync.dma_start(wq_sb[cc][:], wqkvT_d[cc * 128:(cc + 1) * 128, :])
            wprojT_sb = constp.tile([128, NH * C], BF)
            for dc in range(NH):
                nc.sync.dma_start(wprojT_sb[:, dc * C:(dc + 1) * C],
                                  wprojT_d[dc * 128:(dc + 1) * 128, :])
            ebp_sb = [constp.tile([128, 3840], BF, tag=f"ebp{hg}", name=f"ebp{hg}")
                      for hg in range(2)]
            for hg in range(2):
                nc.sync.dma_start(ebp_sb[hg][:], ebp_d[hg])
            g1_sb = constp.tile([128, NHC], FP)
            b1_sb = constp.tile([128, NHC], FP)
            g2_sb = constp.tile([1, C], FP)
            b2_sb = constp.tile([1, C], FP)
            id_sb = constp.tile([128, 128], BF)
            nc.sync.dma_start(g1_sb[:], g1_d[:])
            nc.sync.dma_start(b1_sb[:], b1_d[:])
            nc.sync.dma_start(g2_sb[:], g2_d[:])
            nc.sync.dma_start(b2_sb[:], b2_d[:])
            nc.sync.dma_start(id_sb[:], id_d[:])
            ones_c = constp.tile([128, 1], BF)
            nc.vector.memset(ones_c[:], 1.0)
            ones_rf = constp.tile([128, 128], FP)
            nc.vector.memset(ones_rf[:], 1.0)

            xT_sb = [constp.tile([128, R], BF, tag=f"xT{cc}", name=f"xT{cc}")
                     for cc in range(2)]
            alpha1 = statp.tile([128, NHC], FP)
            beta1 = statp.tile([128, NHC], FP)
            gvsum = statp.tile([128, NH], FP)

            # ========== Phase A: x load/cast, Gram stats, transpose ==========
            with tc.tile_pool(name="xa", bufs=1) as xap, \
                 tc.tile_pool(name="pa", bufs=1, space="PSUM") as pap:
                xb = [xap.tile([128, C], BF, tag=f"xb{rc}", name=f"xb{rc}")
                      for rc in range(NRC)]
                sxxp = [pap.tile([128, C], FP, tag=f"sxx{cc}", name=f"sxx{cc}")
                        for cc in range(2)]
                sxp = pap.tile([128, 2], FP, tag="sxp")
                for rc in range(NRC):
                    nc.sync.dma_start(xb[rc][:], x_d[rc * 128:(rc + 1) * 128, :])
                    for cc in range(2):
                        # Gram + colsum accumulation over all row chunks
                        nc.tensor.matmul(
                            sxxp[cc][:], xb[rc][:, cc * 128:(cc + 1) * 128],
                            xb[rc][:], start=(rc == 0), stop=(rc == NRC - 1))
                        nc.tensor.matmul(
                            sxp[:, cc:cc + 1],
                            xb[rc][:, cc * 128:(cc + 1) * 128], ones_c[:],
                            start=(rc == 0), stop=(rc == NRC - 1))

                # stats: ssum[h] = Wq @ sx ; ssq[h] = diag(Wq Sxx Wq^T)
                with tc.tile_pool(name="st2", bufs=1) as st2p, \
                     tc.tile_pool(name="pt2", bufs=2, space="PSUM") as pt2p, \
                     tc.tile_pool(name="pst", bufs=1, space="PSUM") as pstp:
                    sxx_sb = [st2p.tile([128, C], BF, tag=f"sxs{cc}",
                                        name=f"sxs{cc}") for cc in range(2)]
                    sx_sb = st2p.tile([128, 2], BF)
                    for cc in range(2):
                        nc.vector.tensor_copy(sxx_sb[cc][:], sxxp[cc][:])
                    nc.vector.tensor_copy(sx_sb[:], sxp[:])
                    t1_sb = [st2p.tile([128, H], BF, tag=f"t1{cc}",
                                       name=f"t1{cc}") for cc in range(2)]
                    for cco in range(2):
                        for nb in range(3):
                            t1p = pt2p.tile([128, 512], FP, tag="t1p")
                            for cci in range(2):
                                nc.tensor.matmul(
                                    t1p[:],
                                    sxx_sb[cci][:, cco * 128:(cco + 1) * 128],
                                    wq_sb[cci][:, nb * 512:(nb + 1) * 512],
                                    start=(cci == 0), stop=(cci == 1))
                            nc.vector.tensor_copy(
                                t1_sb[cco][:, nb * 512:(nb + 1) * 512], t1p[:])
                    tt_sb = [st2p.tile([128, H], BF, tag=f"tt{cc}",
                                       name=f"tt{cc}") for cc in range(2)]
                    for cc in range(2):
                        nc.vector.tensor_tensor(tt_sb[cc][:], t1_sb[cc][:],
                                                wq_sb[cc][:], OP.mult)
                    tstat = pstp.tile([128, 2 * NHC], FP)
                    for hc in range(NHC):
                        for cc in range(2):
                            nc.tensor.matmul(
                                tstat[:, hc:hc + 1],
                                wq_sb[cc][:, hc * 128:(hc + 1) * 128],
                                sx_sb[:, cc:cc + 1],
                                start=(cc == 0), stop=(cc == 1))
                            nc.tensor.matmul(
                                tstat[:, NHC + hc:NHC + hc + 1],
                                tt_sb[cc][:, hc * 128:(hc + 1) * 128],
                                ones_c[:], start=(cc == 0), stop=(cc == 1))
                    stats = statp.tile([128, 2 * NHC], FP)
                    nc.vector.tensor_copy(stats[:], tstat[:])

                # transposes run while the stats allreduce is in flight
                with tc.tile_pool(name="pt", bufs=4, space="PSUM") as ptp:
                    for rc in range(NRC):
                        for cc in range(2):
                            pt = ptp.tile([128, 128], BF, tag="pt")
                            nc.tensor.transpose(
                                pt[:], xb[rc][:, cc * 128:(cc + 1) * 128], id_sb[:])
                            nc.vector.tensor_copy(
                                xT_sb[cc][:, rc * 128:(rc + 1) * 128], pt[:])

            # allreduce BN1 stats (overlaps nothing PE-critical yet: issued early)
            bounce_i = dramp.tile([128, 2 * NHC], FP, tag="b1i")
            bounce_o = dramp.tile([128, 2 * NHC], FP, tag="b1o")
            nc.sync.dma_start(bounce_i[:], stats[:])
            nc.gpsimd.collective_compute(
                "AllReduce", OP.add,
                replica_groups=[list(range(NCORES))],
                ins=[bounce_i.opt()], outs=[bounce_o.opt()])
            statsg = statp.tile([128, 2 * NHC], FP)
            nc.sync.dma_start(statsg[:], bounce_o[:])

            mean1 = statp.tile([128, NHC], FP)
            var1 = statp.tile([128, NHC], FP)
            tmp1 = statp.tile([128, NHC], FP)
            nc.vector.tensor_scalar(mean1[:], statsg[:, 0:NHC], 1.0 / NT, None, OP.mult)
            nc.vector.tensor_scalar(var1[:], statsg[:, NHC:2 * NHC], 1.0 / NT, None,
                                    OP.mult)
            nc.vector.tensor_tensor(tmp1[:], mean1[:], mean1[:], OP.mult)
            nc.vector.tensor_tensor(var1[:], var1[:], tmp1[:], OP.subtract)
            nc.vector.tensor_scalar(var1[:], var1[:], EPS, None, OP.add)
            nc.scalar.activation(tmp1[:], var1[:], AF.Ln)
            nc.scalar.activation(var1[:], tmp1[:], AF.Exp, scale=-0.5)   # rstd
            nc.vector.tensor_tensor(alpha1[:], g1_sb[:], var1[:], OP.mult)
            nc.vector.tensor_tensor(beta1[:], mean1[:], alpha1[:], OP.mult)
            nc.vector.tensor_tensor(beta1[:], b1_sb[:], beta1[:], OP.subtract)

            # ========== Phase A2+B: per-batch qkv + attention, pipelined =====
            av_sb = [constp.tile([128, R], BF, tag=f"av{h}", name=f"av{h}")
                     for h in range(NH)]
            with tc.tile_pool(name="qkr", bufs=5) as qkrp, \
                 tc.tile_pool(name="vtr", bufs=5) as vtrp, \
                 tc.tile_pool(name="etp", bufs=6) as etp, \
                 tc.tile_pool(name="rrp", bufs=2) as rrp, \
                 tc.tile_pool(name="pqk", bufs=2, space="PSUM") as pqkp, \
                 tc.tile_pool(name="pav", bufs=2, space="PSUM") as pavp, \
                 tc.tile_pool(name="ps5", bufs=2, space="PSUM") as ps5p:

                qk_t = {}   # (b, hc) -> [128, 320] bf16 (BN1 applied)
                vt_t = {}   # b -> [128, 3072] bf16 (m-part, mc*1024 + dv)

                def qkv_batch(b):
                    vt = vtrp.tile([128, 3072], BF, tag="vt", name="vt")
                    for mc in range(3):
                        ms, mb = MCS[mc], MBS[mc]
                        for dh in range(2):
                            p5 = ps5p.tile([128, 512], FP, tag="p5", name="p5")
                            for cc in range(2):
                                nc.tensor.matmul(
                                    p5[mb:mb + ms, :],
                                    xT_sb[cc][:, b * N + mc * 128:
                                              b * N + mc * 128 + ms],
                                    wq_sb[cc][:, 512 + dh * 512:1024 + dh * 512],
                                    start=(cc == 0), stop=(cc == 1),
                                    tile_position=(0, mb))
                            dst = vt[mb:mb + ms,
                                     mc * 1024 + dh * 512:mc * 1024 + (dh + 1) * 512]
                            if dh == 0:
                                nc.scalar.copy(dst, p5[mb:mb + ms, :])
                            else:
                                nc.vector.tensor_copy(dst, p5[mb:mb + ms, :])
                    vt_t[b] = vt
                    for hc in range(4):
                        p5 = ps5p.tile([128, 512], FP, tag="p5", name="p5")
                        for cc in range(2):
                            nc.tensor.matmul(
                                p5[:, 0:N],
                                wq_sb[cc][:, hc * 128:(hc + 1) * 128],
                                xT_sb[cc][:, b * N:(b + 1) * N],
                                start=(cc == 0), stop=(cc == 1))
                        qt = qkrp.tile([128, N], BF, tag=f"qk{hc}", name="qt")
                        if b < 4:
                            # raw copy (no alpha dep) + in-place BN fixup on
                            # gpsimd (whose queue already waits the allreduce),
                            # so neither the PSUM chain nor DVE stalls on it
                            nc.vector.tensor_copy(qt[:], p5[:, 0:N])
                            nc.gpsimd.tensor_scalar(
                                qt[:], qt[:],
                                alpha1[:, hc:hc + 1], beta1[:, hc:hc + 1],
                                OP.mult, OP.add)
                        else:
                            nc.vector.tensor_scalar(
                                qt[:], p5[:, 0:N],
                                alpha1[:, hc:hc + 1], beta1[:, hc:hc + 1],
                                OP.mult, OP.add)
                        qk_t[(b, hc)] = qt

                def scores(b, hg):
                    # QK matmuls (4-head row-strip packed) + exp + bias mult.
                    # Returns ets[mc] = [128, 1280] bf16 tile (pair p, head j
                    # halves at cols p*640 + j*320).
                    qc = hg
                    ets = []
                    for mc in range(3):
                        ms, mb = MCS[mc], MBS[mc]
                        et = etp.tile([128, 1280], BF, tag="et", name="et")
                        for p in range(2):
                            psq = pqkp.tile([128, 1024], FP, tag="psq",
                                            name="psq")
                            for j in range(2):
                                qr = 32 * (2 * p + j)
                                nc.tensor.matmul(
                                    psq[mb:mb + ms, j * 512:j * 512 + N],
                                    qk_t[(b, 2 + qc)][qr:qr + 32,
                                                      mc * 128:mc * 128 + ms],
                                    qk_t[(b, qc)][qr:qr + 32, :],
                                    tile_position=(qr, mb))
                            src = psq[mb:mb + ms, :].rearrange(
                                "q (two c) -> q two c", two=2)[:, :, 0:N]
                            dst = et[mb:mb + ms,
                                     p * 640:(p + 1) * 640].rearrange(
                                "q (two c) -> q two c", two=2)
                            nc.scalar.activation(dst, src, AF.Exp, scale=SCALE)
                        nc.vector.tensor_tensor(
                            et[mb:mb + ms, :], et[mb:mb + ms, :],
                            ebp_sb[hg][mb:mb + ms, mc * 1280:(mc + 1) * 1280],
                            OP.mult)
                        ets.append(et)
                    return ets

                def softmax_av(b, hg, ets, vt):
                    rs = ps5p.tile([128, 512], FP, tag="p5", name="rs")
                    for mc in range(3):
                        ms, mb = MCS[mc], MBS[mc]
                        for q_ in range(4):
                            et = ets[mc]
                            c0 = (q_ // 2) * 640 + (q_ % 2) * N
                            nc.tensor.matmul(
                                rs[32 * q_:32 * q_ + 1, 0:N],
                                ones_c[mb:mb + ms, 0:1],
                                et[mb:mb + ms, c0:c0 + N],
                                start=(mc == 0), stop=(mc == 2),
                                tile_position=(mb, 32 * q_))
                    rrf = rrp.tile([128, N], FP, tag="rrf", name="rrf")
                    nc.vector.reciprocal_approx_fast(rrf[:], rs[:, 0:N])
                    rbs = {}
                    for q_ in range(4):
                        h = 4 * hg + q_
                        if q_ % 2 == 0:
                            for q2 in (q_, q_ + 1):
                                rb = ps5p.tile([128, 512], FP, tag="p5",
                                               name="rb")
                                nc.tensor.matmul(
                                    rb[:, 0:N], ones_rf[32 * q2:32 * q2 + 1, :],
                                    rrf[32 * q2:32 * q2 + 1, :],
                                    tile_position=(32 * q2, 0))
                                rbs[q2] = rb
                        av = pavp.tile([128, N], FP, tag="av", name="av")
                        for mc in range(3):
                            ms, mb = MCS[mc], MBS[mc]
                            c0 = (q_ // 2) * 640 + (q_ % 2) * N
                            nc.tensor.matmul(
                                av[:],
                                vt[mb:mb + ms, mc * 1024 + h * 128:
                                   mc * 1024 + (h + 1) * 128],
                                ets[mc][mb:mb + ms, c0:c0 + N],
                                start=(mc == 0), stop=(mc == 2),
                                tile_position=(mb, 0))
                        osl = av_sb[h][:, b * N:(b + 1) * N]
                        if q_ % 2 == 0:
                            nc.scalar.copy(osl, av[:])
                        else:
                            nc.vector.tensor_copy(osl, av[:])
                        nc.vector.tensor_tensor(osl, osl, rbs[q_][:, 0:N],
                                                OP.mult)

                def attention(b):
                    vt = vt_t.pop(b)
                    ets0 = scores(b, 0)
                    ets1 = scores(b, 1)
                    softmax_av(b, 0, ets0, vt)
                    softmax_av(b, 1, ets1, vt)
                    for hc in range(4):
                        del qk_t[(b, hc)]

                for b in range(4):
                    qkv_batch(b)
                for b in range(BL):
                    if b + 4 < BL:
                        qkv_batch(b + 4)
                    attention(b)

            # ========== Phase C: gelu, proj, BN2 ==========
            with tc.tile_pool(name="ppy", bufs=4, space="PSUM") as ppy, \
                 tc.tile_pool(name="pst2", bufs=1, space="PSUM") as pst2, \
                 tc.tile_pool(name="yb", bufs=1) as yp, \
                 tc.tile_pool(name="sc3", bufs=4) as scp3:
                # gelu per (head, column-quad) so proj can start after the
                # first quad instead of after all gelu
                gvq = statp.tile([128, NH * 4], FP)
                y16 = yp.tile([128, NRC * C], BF)
                ystat = pst2.tile([1, 2 * C], FP, tag="yst")
                for cq in range(4):
                    for h in range(NH):
                        nc.scalar.activation(
                            av_sb[h][:, cq * 1280:(cq + 1) * 1280],
                            av_sb[h][:, cq * 1280:(cq + 1) * 1280], AF.Gelu,
                            scale=alpha1[:, 4 + h:5 + h],
                            bias=beta1[:, 4 + h:5 + h],
                            accum_out=gvq[:, 4 * h + cq:4 * h + cq + 1])
                    for i in range(cq * 10, cq * 10 + 10):
                        py = ppy.tile([128, C], FP, tag="py")
                        for h in range(NH):
                            nc.tensor.matmul(
                                py[:], av_sb[h][:, i * 128:(i + 1) * 128],
                                wprojT_sb[:, h * C:(h + 1) * C],
                                start=(h == 0), stop=(h == NH - 1))
                        ysl = y16[:, i * C:(i + 1) * C]
                        nc.vector.tensor_copy(ysl, py[:])
                        yq = scp3.tile([128, C], BF, tag="yq")
                        nc.vector.tensor_tensor(yq[:], ysl, ysl, OP.mult)
                        nc.tensor.matmul(ystat[0:1, C:2 * C], ones_c[:, 0:1],
                                         yq[:],
                                         start=(i == 0), stop=(i == NRC - 1))
                for h in range(NH):
                    nc.vector.tensor_reduce(
                        gvsum[:, h:h + 1], gvq[:, 4 * h:4 * h + 4],
                        mybir.AxisListType.X, OP.add)
                gvs16 = statp.tile([128, NH], BF)
                nc.vector.tensor_copy(gvs16[:], gvsum[:])
                for h in range(NH):
                    nc.tensor.matmul(ystat[0:1, 0:C], gvs16[:, h:h + 1],
                                     wprojT_sb[:, h * C:(h + 1) * C],
                                     start=(h == 0), stop=(h == NH - 1))

                st2 = statp.tile([1, 2 * C], FP)
                nc.vector.tensor_copy(st2[:], ystat[:])
                b2i = dramp.tile([1, 2 * C], FP, tag="b2i")
                b2o = dramp.tile([1, 2 * C], FP, tag="b2o")
                nc.sync.dma_start(b2i[:], st2[:])
                nc.gpsimd.collective_compute(
                    "AllReduce", OP.add,
                    replica_groups=[list(range(NCORES))],
                    ins=[b2i.opt()], outs=[b2o.opt()])
                st2g = statp.tile([1, 2 * C], FP)
                nc.sync.dma_start(st2g[:], b2o[:])

                mean2 = statp.tile([1, C], FP)
                var2 = statp.tile([1, C], FP)
                tmp2 = statp.tile([1, C], FP)
                alpha2 = statp.tile([1, C], FP)
                beta2 = statp.tile([1, C], FP)
                nc.vector.tensor_scalar(mean2[:], st2g[:, 0:C], 1.0 / NT, None, OP.mult)
                nc.vector.tensor_scalar(var2[:], st2g[:, C:2 * C], 1.0 / NT, None,
                                        OP.mult)
                nc.vector.tensor_tensor(tmp2[:], mean2[:], mean2[:], OP.mult)
                nc.vector.tensor_tensor(var2[:], var2[:], tmp2[:], OP.subtract)
                nc.vector.tensor_scalar(var2[:], var2[:], EPS, None, OP.add)
                nc.scalar.activation(tmp2[:], var2[:], AF.Ln)
                nc.scalar.activation(var2[:], tmp2[:], AF.Exp, scale=-0.5)  # rstd2
                nc.vector.tensor_tensor(alpha2[:], g2_sb[:], var2[:], OP.mult)
                nc.vector.tensor_tensor(beta2[:], mean2[:], alpha2[:], OP.mult)
                nc.vector.tensor_tensor(beta2[:], b2_sb[:], beta2[:], OP.subtract)

                a2ps = pst2.tile([128, C], FP, tag="a2ps")
                b2ps = pst2.tile([128, C], FP, tag="b2ps")
                nc.tensor.matmul(a2ps[:], ones_rf[0:1, :], alpha2[:])
                nc.tensor.matmul(b2ps[:], ones_rf[0:1, :], beta2[:])
                a2bc = statp.tile([128, C], FP)
                b2bc = statp.tile([128, C], FP)
                nc.vector.tensor_copy(a2bc[:], a2ps[:])
                nc.vector.tensor_copy(b2bc[:], b2ps[:])

                for i in range(NRC):
                    yo = scp3.tile([128, C], FP, tag="yo")
                    nc.vector.tensor_tensor(yo[:], y16[:, i * C:(i + 1) * C],
                                            a2bc[:], OP.mult)
                    nc.vector.tensor_tensor(yo[:], yo[:], b2bc[:], OP.add)
                    nc.sync.dma_start(y_d[i * 128:(i + 1) * 128, :], yo[:])

    nc.compile()
    return nc


_PROG = None


def _get_prog():
    global _PROG
    if _PROG is None:
        _PROG = build_program()
    return _PROG


def _host_prep(x, Wqkv, g1, b1, ab, Wproj, g2, b2, idxs):
    perm = np.empty(H, dtype=np.int64)
    for h in range(NH):
        base = h * (2 * DK + DV)
        perm[DK * h: DK * (h + 1)] = np.arange(base, base + DK)
        perm[NH * DK + DK * h: NH * DK + DK * (h + 1)] = \
            np.arange(base + DK, base + 2 * DK)
        perm[2 * NH * DK + DV * h: 2 * NH * DK + DV * (h + 1)] = \
            np.arange(base + 2 * DK, base + 2 * DK + DV)
    x = np.asarray(x, dtype=np.float32)
    Wqkv = np.asarray(Wqkv, dtype=np.float32)
    wqkvT = np.ascontiguousarray(Wqkv[perm, :].T).astype(ml_dtypes.bfloat16)
    g1c = np.ascontiguousarray(np.asarray(g1, np.float32)[perm].reshape(NHC, 128).T)
    b1c = np.ascontiguousarray(np.asarray(b1, np.float32)[perm].reshape(NHC, 128).T)
    wprojT = np.ascontiguousarray(np.asarray(Wproj, np.float32).T).astype(
        ml_dtypes.bfloat16)                                            # (1024, 256)
    eb = np.exp(np.asarray(ab, np.float32))[:, np.asarray(idxs)]       # (8,320,320)
    ebp = np.zeros((2, 128, 3840), np.float32)
    for hg in range(2):
        for mc in range(3):
            ms, mb = MCS[mc], MBS[mc]
            for p in range(2):
                for j in range(2):
                    h = 4 * hg + 2 * p + j
                    c0 = mc * 1280 + p * 640 + j * 320
                    ebp[hg, mb:mb + ms, c0:c0 + N] = \
                        eb[h, mc * 128:mc * 128 + ms, :]
    common = {
        "wqkvT": wqkvT, "wprojT": wprojT,
        "ebp": ebp.astype(ml_dtypes.bfloat16),
        "g1c": g1c, "b1c": b1c,
        "g2": np.asarray(g2, np.float32).reshape(1, C),
        "b2": np.asarray(b2, np.float32).reshape(1, C),
        "ident": np.eye(128, dtype=np.float32).astype(ml_dtypes.bfloat16),
    }
    in_maps = []
    for c in range(NCORES):
        m = dict(common)
        m["x"] = np.ascontiguousarray(x[c * BL:(c + 1) * BL].reshape(R, C)).astype(
            ml_dtypes.bfloat16)
        in_maps.append(m)
    return in_maps


def _run(in_maps, trace=False):
    nc = _get_prog()
    res = run_bass_kernel_spmd(nc, in_maps, core_ids=list(range(NCORES)),
                               trace=trace)
    out = np.concatenate(
        [np.asarray(res.results[c]["y"]).reshape(BL, N, C) for c in range(NCORES)],
        axis=0)
    return out.astype(np.float32), res


def kernel(**inputs):
    out, _ = _run(_host_prep(**inputs))
    return out


def run_traced(**inputs):
    return _run(_host_prep(**inputs), trace=True)


# revision 26
# speedup vs baseline: 1.2871x; 1.0511x over previous
"""Trainium2 Bass kernel for nn_Attention (qkv+BN -> biased softmax attention -> gelu -> proj+BN).

Sharding: data-parallel over batch B=128 across 8 NeuronCores (16 batches each).
BatchNorm (training-mode) statistics all-reduced across cores (tiny collectives).

v2 design (vs baseline):
  - BN1 stats from x-Gram matrix (Sxx = x^T x, sx = sum x) so the stats
    allreduce is issued early and overlaps the qkv matmul; BN1 q/k affine is
    folded into the PSUM->SBUF copy.
  - v computed directly transposed ([m, dv] per batch) -> no DMA transposes.
  - attention scores 4-head-packed via tile_position (contraction 32 row
    strips); rowsums 4-head col-strip packed; exp over PSUM pairs with one
    strided-AP activation; bias multiply exp(s)*exp(b) on DVE in bf16.
  - softmax reciprocal broadcast via stride-0-partition DMA (no PE bcast).
  - per-batch pipeline: qkv(b+2) issued while attention(b) runs.
  - gelu folds BN1-v (attn rows sum to 1); proj + BN2 as final phase.
"""
import os
import numpy as np
import ml_dtypes

import concourse.bass as bass
import concourse.tile as tile
from concourse import bacc, mybir
from concourse.bass_utils import run_bass_kernel_spmd

NCORES = int(os.environ.get("KERN_NCORES", "8"))
TRACE_SIM = os.environ.get("KERN_TRACE_SIM", "") == "1"
B, N, C = 128, 320, 256
NH, DK, DV = 8, 32, 128
H = NH * (2 * DK + DV)       # 1536
DH = NH * DV                 # 1024
BL = B // 8                  # 16 batches/core
R = BL * N                   # 5120 rows/core
NT = B * N                   # 40960 global rows
EPS = 1e-5
SCALE = DK ** -0.5
FP = mybir.dt.float32
BF = mybir.dt.bfloat16

NHC = H // 128               # 12 h-chunks (0-1 q, 2-3 k, 4-11 v)
NRC = R // 128               # 40 row chunks
MCS = [128, 128, 64]         # chunking of N=320 into m-chunks
MBS = [0, 0, 64]             # partition base per m-chunk
AF = mybir.ActivationFunctionType
OP = mybir.AluOpType


def build_program():
    nc = bacc.Bacc("TRN2", target_bir_lowering=False, debug=False,
                   enable_asserts=False, num_devices=NCORES)
    x_d = nc.dram_tensor("x", [R, C], BF, kind="ExternalInput").ap()
    wqkvT_d = nc.dram_tensor("wqkvT", [C, H], BF, kind="ExternalInput").ap()
    wprojT_d = nc.dram_tensor("wprojT", [DH, C], BF, kind="ExternalInput").ap()
    ebp_d = nc.dram_tensor("ebp", [2, 128, 3840], BF, kind="ExternalInput").ap()
    g1_d = nc.dram_tensor("g1c", [128, NHC], FP, kind="ExternalInput").ap()
    b1_d = nc.dram_tensor("b1c", [128, NHC], FP, kind="ExternalInput").ap()
    g2_d = nc.dram_tensor("g2", [1, C], FP, kind="ExternalInput").ap()
    b2_d = nc.dram_tensor("b2", [1, C], FP, kind="ExternalInput").ap()
    id_d = nc.dram_tensor("ident", [128, 128], BF, kind="ExternalInput").ap()
    y_d = nc.dram_tensor("y", [R, C], FP, kind="ExternalOutput").ap()

    with tile.TileContext(nc, trace_sim=TRACE_SIM) as tc:
        with tc.tile_pool(name="const", bufs=1) as constp, \
             tc.tile_pool(name="dram", bufs=1, space="DRAM") as dramp, \
             tc.tile_pool(name="stat", bufs=1) as statp:

            # ---- constants ----
            wq_sb = [constp.tile([128, H], BF, tag=f"wq{cc}", name=f"wq{cc}")
                     for cc in range(2)]
            for cc in range(2):
                nc.sync.dma_start(wq_sb[cc][:], wqkvT_d[cc * 128:(cc + 1) * 128, :])
            wprojT_sb = constp.tile([128, NH * C], BF)
            for dc in range(NH):
                nc.sync.dma_start(wprojT_sb[:, dc * C:(dc + 1) * C],
                                  wprojT_d[dc * 128:(dc + 1) * 128, :])
            ebp_sb = [constp.tile([128, 3840], BF, tag=f"ebp{hg}", name=f"ebp{hg}")
                      for hg in range(2)]
            for hg in range(2):
                nc.sync.dma_start(ebp_sb[hg][:], ebp_d[hg])
            g1_sb = constp.tile([128, NHC], FP)
            b1_sb = constp.tile([128, NHC], FP)
            g2_sb = constp.tile([1, C], FP)
            b2_sb = constp.tile([1, C], FP)
            id_sb = constp.tile([128, 128], BF)
            nc.sync.dma_start(g1_sb[:], g1_d[:])
            nc.sync.dma_start(b1_sb[:], b1_d[:])
            nc.sync.dma_start(g2_sb[:], g2_d[:])
            nc.sync.dma_start(b2_sb[:], b2_d[:])
            nc.sync.dma_start(id_sb[:], id_d[:])
            ones_c = constp.tile([128, 1], BF)
            nc.vector.memset(ones_c[:], 1.0)
            ones_rf = constp.tile([128, 128], FP)
            nc.vector.memset(ones_rf[:], 1.0)

            xT_sb = [constp.tile([128, R], BF, tag=f"xT{cc}", name=f"xT{cc}")
                     for cc in range(2)]
            alpha1 = statp.tile([128, NHC], FP)
            beta1 = statp.tile([128, NHC], FP)
            gvsum = statp.tile([128, NH], FP)

            # ========== Phase A: x load/cast, Gram stats, transpose ==========
            with tc.tile_pool(name="xa", bufs=1) as xap, \
                 tc.tile_pool(name="pa", bufs=1, space="PSUM") as pap:
                xb = [xap.tile([128, C], BF, tag=f"xb{rc}", name=f"xb{rc}")
                      for rc in range(NRC)]
                sxxp = [pap.tile([128, C], FP, tag=f"sxx{cc}", name=f"sxx{cc}")
                        for cc in range(2)]
                sxp = pap.tile([128, 2], FP, tag="sxp")
                for rc in range(NRC):
                    nc.sync.dma_start(xb[rc][:], x_d[rc * 128:(rc + 1) * 128, :])
                    for cc in range(2):
                        # Gram + colsum accumulation over all row chunks
                        nc.tensor.matmul(
                            sxxp[cc][:], xb[rc][:, cc * 128:(cc + 1) * 128],
                            xb[rc][:], start=(rc == 0), stop=(rc == NRC - 1))
                        nc.tensor.matmul(
                            sxp[:, cc:cc + 1],
                            xb[rc][:, cc * 128:(cc + 1) * 128], ones_c[:],
                            start=(rc == 0), stop=(rc == NRC - 1))

                # stats: ssum[h] = Wq @ sx ; ssq[h] = diag(Wq Sxx Wq^T)
                with tc.tile_pool(name="st2", bufs=1) as st2p, \
                     tc.tile_pool(name="pt2", bufs=2, space="PSUM") as pt2p, \
                     tc.tile_pool(name="pst", bufs=1, space="PSUM") as pstp:
                    sxx_sb = [st2p.tile([128, C], BF, tag=f"sxs{cc}",
                                        name=f"sxs{cc}") for cc in range(2)]
                    sx_sb = st2p.tile([128, 2], BF)
                    for cc in range(2):
                        nc.vector.tensor_copy(sxx_sb[cc][:], sxxp[cc][:])
                    nc.vector.tensor_copy(sx_sb[:], sxp[:])
                    t1_sb = [st2p.tile([128, H], BF, tag=f"t1{cc}",
                                       name=f"t1{cc}") for cc in range(2)]
                    for cco in range(2):
                        for nb in range(3):
                            t1p = pt2p.tile([128, 512], FP, tag="t1p")
                            for cci in range(2):
                                nc.tensor.matmul(
                                    t1p[:],
                                    sxx_sb[cci][:, cco * 128:(cco + 1) * 128],
                                    wq_sb[cci][:, nb * 512:(nb + 1) * 512],
                                    start=(cci == 0), stop=(cci == 1))
                            nc.vector.tensor_copy(
                                t1_sb[cco][:, nb * 512:(nb + 1) * 512], t1p[:])
                    tt_sb = [st2p.tile([128, H], BF, tag=f"tt{cc}",
                                       name=f"tt{cc}") for cc in range(2)]
                    for cc in range(2):
                        nc.vector.tensor_tensor(tt_sb[cc][:], t1_sb[cc][:],
                                                wq_sb[cc][:], OP.mult)
                    tstat = pstp.tile([128, 2 * NHC], FP)
                    for hc in range(NHC):
                        for cc in range(2):
                            nc.tensor.matmul(
                                tstat[:, hc:hc + 1],
                                wq_sb[cc][:, hc * 128:(hc + 1) * 128],
                                sx_sb[:, cc:cc + 1],
                                start=(cc == 0), stop=(cc == 1))
                            nc.tensor.matmul(
                                tstat[:, NHC + hc:NHC + hc + 1],
                                tt_sb[cc][:, hc * 128:(hc + 1) * 128],
                                ones_c[:], start=(cc == 0), stop=(cc == 1))
                    stats = statp.tile([128, 2 * NHC], FP)
                    nc.vector.tensor_copy(stats[:], tstat[:])

                # transposes run while the stats allreduce is in flight
                with tc.tile_pool(name="pt", bufs=4, space="PSUM") as ptp:
                    for rc in range(NRC):
                        for cc in range(2):
                            pt = ptp.tile([128, 128], BF, tag="pt")
                            nc.tensor.transpose(
                                pt[:], xb[rc][:, cc * 128:(cc + 1) * 128], id_sb[:])
                            nc.vector.tensor_copy(
                                xT_sb[cc][:, rc * 128:(rc + 1) * 128], pt[:])

            # allreduce BN1 stats (overlaps nothing PE-critical yet: issued early)
            bounce_i = dramp.tile([128, 2 * NHC], FP, tag="b1i")
            bounce_o = dramp.tile([128, 2 * NHC], FP, tag="b1o")
            nc.sync.dma_start(bounce_i[:], stats[:])
            nc.gpsimd.collective_compute(
                "AllReduce", OP.add,
                replica_groups=[list(range(NCORES))],
                ins=[bounce_i.opt()], outs=[bounce_o.opt()])
            statsg = statp.tile([128, 2 * NHC], FP)
            nc.sync.dma_start(statsg[:], bounce_o[:])

            mean1 = statp.tile([128, NHC], FP)
            var1 = statp.tile([128, NHC], FP)
            tmp1 = statp.tile([128, NHC], FP)
            nc.vector.tensor_scalar(mean1[:], statsg[:, 0:NHC], 1.0 / NT, None, OP.mult)
            nc.vector.tensor_scalar(var1[:], statsg[:, NHC:2 * NHC], 1.0 / NT, None,
                                    OP.mult)
            nc.vector.tensor_tensor(tmp1[:], mean1[:], mean1[:], OP.mult)
            nc.vector.tensor_tensor(var1[:], var1[:], tmp1[:], OP.subtract)
            nc.vector.tensor_scalar(var1[:], var1[:], EPS, None, OP.add)
            nc.scalar.activation(tmp1[:], var1[:], AF.Ln)
            nc.scalar.activation(var1[:], tmp1[:], AF.Exp, scale=-0.5)   # rstd
            nc.vector.tensor_tensor(alpha1[:], g1_sb[:], var1[:], OP.mult)
            nc.vector.tensor_tensor(beta1[:], mean1[:], alpha1[:], OP.mult)
            nc.vector.tensor_tensor(beta1[:], b1_sb[:], beta1[:], OP.subtract)

            # ========== Phase A2+B: per-batch qkv + attention, pipelined =====
            av_sb = [constp.tile([128, R], BF, tag=f"av{h}", name=f"av{h}")
                     for h in range(NH)]
            with tc.tile_pool(name="qkr", bufs=5) as qkrp, \
                 tc.tile_pool(name="vtr", bufs=5) as vtrp, \
                 tc.tile_pool(name="etp", bufs=6) as etp, \
                 tc.tile_pool(name="rrp", bufs=2) as rrp, \
                 tc.tile_pool(name="pqk", bufs=3, space="PSUM") as pqkp, \
                 tc.tile_pool(name="pav", bufs=2, space="PSUM") as pavp:

                qk_t = {}   # (b, hc) -> [128, 320] bf16 (BN1 applied)
                vt_t = {}   # b -> [128, 3072] bf16 (m-part, mc*1024 + dv)

                def qkv_batch(b):
                    vt = vtrp.tile([128, 3072], BF, tag="vt", name="vt")
                    for mc in range(3):
                        ms, mb = MCS[mc], MBS[mc]
                        for dh in range(2):
                            p5 = pqkp.tile([128, 1024], FP, tag="psq", name="p5")
                            for cc in range(2):
                                nc.tensor.matmul(
                                    p5[mb:mb + ms, 0:512],
                                    xT_sb[cc][:, b * N + mc * 128:
                                              b * N + mc * 128 + ms],
                                    wq_sb[cc][:, 512 + dh * 512:1024 + dh * 512],
                                    start=(cc == 0), stop=(cc == 1),
                                    tile_position=(0, mb))
                            dst = vt[mb:mb + ms,
                                     mc * 1024 + dh * 512:mc * 1024 + (dh + 1) * 512]
                            if dh == 0:
                                nc.scalar.copy(dst, p5[mb:mb + ms, 0:512])
                            else:
                                nc.vector.tensor_copy(dst, p5[mb:mb + ms, 0:512])
                    vt_t[b] = vt
                    for hc in range(4):
                        p5 = pqkp.tile([128, 1024], FP, tag="psq", name="p5")
                        for cc in range(2):
                            nc.tensor.matmul(
                                p5[:, 0:N],
                                wq_sb[cc][:, hc * 128:(hc + 1) * 128],
                                xT_sb[cc][:, b * N:(b + 1) * N],
                                start=(cc == 0), stop=(cc == 1))
                        qt = qkrp.tile([128, N], BF, tag=f"qk{hc}", name="qt")
                        if b < 4:
                            # raw copy (no alpha dep) + in-place BN fixup on
                            # gpsimd (whose queue already waits the allreduce),
                            # so neither the PSUM chain nor DVE stalls on it
                            nc.vector.tensor_copy(qt[:], p5[:, 0:N])
                            nc.gpsimd.tensor_scalar(
                                qt[:], qt[:],
                                alpha1[:, hc:hc + 1], beta1[:, hc:hc + 1],
                                OP.mult, OP.add)
                        else:
                            nc.vector.tensor_scalar(
                                qt[:], p5[:, 0:N],
                                alpha1[:, hc:hc + 1], beta1[:, hc:hc + 1],
                                OP.mult, OP.add)
                        qk_t[(b, hc)] = qt

                def scores(b, hg):
                    # QK matmuls (4-head row-strip packed) + exp + bias mult.
                    # Returns ets[mc] = [128, 1280] bf16 tile (pair p, head j
                    # halves at cols p*640 + j*320).
                    qc = hg
                    ets = []
                    for mc in range(3):
                        ms, mb = MCS[mc], MBS[mc]
                        et = etp.tile([128, 1280], BF, tag="et", name="et")
                        for p in range(2):
                            psq = pqkp.tile([128, 1024], FP, tag="psq",
                                            name="psq")
                            for j in range(2):
                                qr = 32 * (2 * p + j)
                                nc.tensor.matmul(
                                    psq[mb:mb + ms, j * 512:j * 512 + N],
                                    qk_t[(b, 2 + qc)][qr:qr + 32,
                                                      mc * 128:mc * 128 + ms],
                                    qk_t[(b, qc)][qr:qr + 32, :],
                                    tile_position=(qr, mb))
                            src = psq[mb:mb + ms, :].rearrange(
                                "q (two c) -> q two c", two=2)[:, :, 0:N]
                            dst = et[mb:mb + ms,
                                     p * 640:(p + 1) * 640].rearrange(
                                "q (two c) -> q two c", two=2)
                            nc.scalar.activation(dst, src, AF.Exp, scale=SCALE)
                        nc.vector.tensor_tensor(
                            et[mb:mb + ms, :], et[mb:mb + ms, :],
                            ebp_sb[hg][mb:mb + ms, mc * 1280:(mc + 1) * 1280],
                            OP.mult)
                        ets.append(et)
                    return ets

                def softmax_av(b, hg, ets, vt):
                    rs = pqkp.tile([128, 1024], FP, tag="psq", name="rs")
                    for mc in range(3):
                        ms, mb = MCS[mc], MBS[mc]
                        for q_ in range(4):
                            et = ets[mc]
                            c0 = (q_ // 2) * 640 + (q_ % 2) * N
                            nc.tensor.matmul(
                                rs[32 * q_:32 * q_ + 1, 0:N],
                                ones_c[mb:mb + ms, 0:1],
                                et[mb:mb + ms, c0:c0 + N],
                                start=(mc == 0), stop=(mc == 2),
                                tile_position=(mb, 32 * q_))
                    rrf = rrp.tile([128, N], FP, tag="rrf", name="rrf")
                    nc.vector.reciprocal_approx_fast(rrf[:], rs[:, 0:N])
                    rbs = {}
                    for q_ in range(4):
                        h = 4 * hg + q_
                        if q_ % 2 == 0:
                            rbt = pqkp.tile([128, 1024], FP, tag="psq",
                                            name="rbt")
                            for k2, q2 in enumerate((q_, q_ + 1)):
                                nc.tensor.matmul(
                                    rbt[:, k2 * 512:k2 * 512 + N],
                                    ones_rf[32 * q2:32 * q2 + 1, :],
                                    rrf[32 * q2:32 * q2 + 1, :],
                                    tile_position=(32 * q2, 0))
                                rbs[q2] = rbt[:, k2 * 512:k2 * 512 + N]
                        av = pavp.tile([128, N], FP, tag="av", name="av")
                        for mc in range(3):
                            ms, mb = MCS[mc], MBS[mc]
                            c0 = (q_ // 2) * 640 + (q_ % 2) * N
                            nc.tensor.matmul(
                                av[:],
                                vt[mb:mb + ms, mc * 1024 + h * 128:
                                   mc * 1024 + (h + 1) * 128],
                                ets[mc][mb:mb + ms, c0:c0 + N],
                                start=(mc == 0), stop=(mc == 2),
                                tile_position=(mb, 0))
                        osl = av_sb[h][:, b * N:(b + 1) * N]
                        if q_ % 2 == 0:
                            nc.scalar.copy(osl, av[:])
                        else:
                            nc.vector.tensor_copy(osl, av[:])
                        nc.vector.tensor_tensor(osl, osl, rbs[q_], OP.mult)

                def attention(b):
                    vt = vt_t.pop(b)
                    ets0 = scores(b, 0)
                    ets1 = scores(b, 1)
                    softmax_av(b, 0, ets0, vt)
                    softmax_av(b, 1, ets1, vt)
                    for hc in range(4):
                        del qk_t[(b, hc)]

                for b in range(4):
                    qkv_batch(b)
                for b in range(BL):
                    if b + 4 < BL:
                        qkv_batch(b + 4)
                    attention(b)

            # ========== Phase C: gelu, proj, BN2 ==========
            with tc.tile_pool(name="ppy", bufs=4, space="PSUM") as ppy, \
                 tc.tile_pool(name="pst2", bufs=1, space="PSUM") as pst2, \
                 tc.tile_pool(name="yb", bufs=1) as yp, \
                 tc.tile_pool(name="sc3", bufs=4) as scp3:
                # gelu per (head, column-quad) so proj can start after the
                # first quad instead of after all gelu
                gvq = statp.tile([128, NH * 4], FP)
                y16 = yp.tile([128, NRC * C], BF)
                ystat = pst2.tile([1, 2 * C], FP, tag="yst")
                for cq in range(4):
                    for h in range(NH):
                        nc.scalar.activation(
                            av_sb[h][:, cq * 1280:(cq + 1) * 1280],
                            av_sb[h][:, cq * 1280:(cq + 1) * 1280], AF.Gelu,
                            scale=alpha1[:, 4 + h:5 + h],
                            bias=beta1[:, 4 + h:5 + h],
                            accum_out=gvq[:, 4 * h + cq:4 * h + cq + 1])
                    for i in range(cq * 10, cq * 10 + 10):
                        py = ppy.tile([128, C], FP, tag="py")
                        for h in range(NH):
                            nc.tensor.matmul(
                                py[:], av_sb[h][:, i * 128:(i + 1) * 128],
                                wprojT_sb[:, h * C:(h + 1) * C],
                                start=(h == 0), stop=(h == NH - 1))
                        ysl = y16[:, i * C:(i + 1) * C]
                        nc.vector.tensor_copy(ysl, py[:])
                        yq = scp3.tile([128, C], BF, tag="yq")
                        nc.vector.tensor_tensor(yq[:], ysl, ysl, OP.mult)
                        nc.tensor.matmul(ystat[0:1, C:2 * C], ones_c[:, 0:1],
                                         yq[:],
                                         start=(i == 0), stop=(i == NRC - 1))
                for h in range(NH):
                    nc.vector.tensor_reduce(
                        gvsum[:, h:h + 1], gvq[:, 4 * h:4 * h + 4],
                        mybir.AxisListType.X, OP.add)
                gvs16 = statp.tile([128, NH], BF)
                nc.vector.tensor_copy(gvs16[:], gvsum[:])
                for h in range(NH):
                    nc.tensor.matmul(ystat[0:1, 0:C], gvs16[:, h:h + 1],
                                     wprojT_sb[:, h * C:(h + 1) * C],
                                     start=(h == 0), stop=(h == NH - 1))

                st2 = statp.tile([1, 2 * C], FP)
                nc.vector.tensor_copy(st2[:], ystat[:])
                b2i = dramp.tile([1, 2 * C], FP, tag="b2i")
                b2o = dramp.tile([1, 2 * C], FP, tag="b2o")
                nc.sync.dma_start(b2i[:], st2[:])
                nc.gpsimd.collective_compute(
                    "AllReduce", OP.add,
                    replica_groups=[list(range(NCORES))],
                    ins=[b2i.opt()], outs=[b2o.opt()])
                st2g = statp.tile([1, 2 * C], FP)
                nc.sync.dma_start(st2g[:], b2o[:])

                mean2 = statp.tile([1, C], FP)
                var2 = statp.tile([1, C], FP)
                tmp2 = statp.tile([1, C], FP)
                alpha2 = statp.tile([1, C], FP)
                beta2 = statp.tile([1, C], FP)
                nc.vector.tensor_scalar(mean2[:], st2g[:, 0:C], 1.0 / NT, None, OP.mult)
                nc.vector.tensor_scalar(var2[:], st2g[:, C:2 * C], 1.0 / NT, None,
                                        OP.mult)
                nc.vector.tensor_tensor(tmp2[:], mean2[:], mean2[:], OP.mult)
                nc.vector.tensor_tensor(var2[:], var2[:], tmp2[:], OP.subtract)
                nc.vector.tensor_scalar(var2[:], var2[:], EPS, None, OP.add)
                nc.scalar.activation(tmp2[:], var2[:], AF.Ln)
                nc.scalar.activation(var2[:], tmp2[:], AF.Exp, scale=-0.5)  # rstd2
                nc.vector.tensor_tensor(alpha2[:], g2_sb[:], var2[:], OP.mult)
                nc.vector.tensor_tensor(beta2[:], mean2[:], alpha2[:], OP.mult)
                nc.vector.tensor_tensor(beta2[:], b2_sb[:], beta2[:], OP.subtract)

                a2ps = pst2.tile([128, C], FP, tag="a2ps")
                b2ps = pst2.tile([128, C], FP, tag="b2ps")
                nc.tensor.matmul(a2ps[:], ones_rf[0:1, :], alpha2[:])
                nc.tensor.matmul(b2ps[:], ones_rf[0:1, :], beta2[:])
                a2bc = statp.tile([128, C], FP)
                b2bc = statp.tile([128, C], FP)
                nc.vector.tensor_copy(a2bc[:], a2ps[:])
                nc.vector.tensor_copy(b2bc[:], b2ps[:])

                for i in range(NRC):
                    yo = scp3.tile([128, C], FP, tag="yo")
                    nc.vector.tensor_tensor(yo[:], y16[:, i * C:(i + 1) * C],
                                            a2bc[:], OP.mult)
                    nc.vector.tensor_tensor(yo[:], yo[:], b2bc[:], OP.add)
                    nc.sync.dma_start(y_d[i * 128:(i + 1) * 128, :], yo[:])

    nc.compile()
    return nc


_PROG = None


def _get_prog():
    global _PROG
    if _PROG is None:
        _PROG = build_program()
    return _PROG


def _host_prep(x, Wqkv, g1, b1, ab, Wproj, g2, b2, idxs):
    perm = np.empty(H, dtype=np.int64)
    for h in range(NH):
        base = h * (2 * DK + DV)
        perm[DK * h: DK * (h + 1)] = np.arange(base, base + DK)
        perm[NH * DK + DK * h: NH * DK + DK * (h + 1)] = \
            np.arange(base + DK, base + 2 * DK)
        perm[2 * NH * DK + DV * h: 2 * NH * DK + DV * (h + 1)] = \
            np.arange(base + 2 * DK, base + 2 * DK + DV)
    x = np.asarray(x, dtype=np.float32)
    Wqkv = np.asarray(Wqkv, dtype=np.float32)
    wqkvT = np.ascontiguousarray(Wqkv[perm, :].T).astype(ml_dtypes.bfloat16)
    g1c = np.ascontiguousarray(np.asarray(g1, np.float32)[perm].reshape(NHC, 128).T)
    b1c = np.ascontiguousarray(np.asarray(b1, np.float32)[perm].reshape(NHC, 128).T)
    wprojT = np.ascontiguousarray(np.asarray(Wproj, np.float32).T).astype(
        ml_dtypes.bfloat16)                                            # (1024, 256)
    eb = np.exp(np.asarray(ab, np.float32))[:, np.asarray(idxs)]       # (8,320,320)
    ebp = np.zeros((2, 128, 3840), np.float32)
    for hg in range(2):
        for mc in range(3):
            ms, mb = MCS[mc], MBS[mc]
            for p in range(2):
                for j in range(2):
                    h = 4 * hg + 2 * p + j
                    c0 = mc * 1280 + p * 640 + j * 320
                    ebp[hg, mb:mb + ms, c0:c0 + N] = \
                        eb[h, mc * 128:mc * 128 + ms, :]
    common = {
        "wqkvT": wqkvT, "wprojT": wprojT,
        "ebp": ebp.astype(ml_dtypes.bfloat16),
        "g1c": g1c, "b1c": b1c,
        "g2": np.asarray(g2, np.float32).reshape(1, C),
        "b2": np.asarray(b2, np.float32).reshape(1, C),
        "ident": np.eye(128, dtype=np.float32).astype(ml_dtypes.bfloat16),
    }
    in_maps = []
    for c in range(NCORES):
        m = dict(common)
        m["x"] = np.ascontiguousarray(x[c * BL:(c + 1) * BL].reshape(R, C)).astype(
            ml_dtypes.bfloat16)
        in_maps.append(m)
    return in_maps


def _run(in_maps, trace=False):
    nc = _get_prog()
    res = run_bass_kernel_spmd(nc, in_maps, core_ids=list(range(NCORES)),
                               trace=trace)
    out = np.concatenate(
        [np.asarray(res.results[c]["y"]).reshape(BL, N, C) for c in range(NCORES)],
        axis=0)
    return out.astype(np.float32), res


def kernel(**inputs):
    out, _ = _run(_host_prep(**inputs))
    return out


def run_traced(**inputs):
    return _run(_host_prep(**inputs), trace=True)
